# revision 32
# baseline (speedup 1.0000x reference)
"""Trainium2 Bass kernel for nn_Blur (upfirdn2d: up=2, pad=(2,1,2,1), 4-tap
separable filter [1,3,3,1] x [1,3,3,1] / 64).

Input  x [16, 128, 128, 128] f32  ->  Output [16, 128, 256, 256] f32.

Math (polyphase decomposition of the zero-insertion upsample + conv):
  per axis, even outputs:  y[2i]   = (1*x[i-1] + 3*x[i]) / 8
            odd  outputs:  y[2i+1] = (3*x[i]   + 1*x[i+1]) / 8

The kernel is HBM-bandwidth bound (the 16 per-core DMA engines cap at
~360 GB/s aggregate), so all device I/O is fp16 (taps 1/64, 3/64 are
exact in fp16; quantization error ~4e-4 rel, well inside the 2e-2
gate): 8.4 MB in + 33.5 MB out per core, ~131 us of DMA-engine packet
time.

DVE runs its 2x fast modes only when every operand is 2-byte,
SBUF-resident, and packed (innermost stride 1) — interleaved stride-2
column writes disqualify it and cost ~1.2-1.4 ns/elem. So the
horizontal pass writes PLANAR column phases (even plane
E[j]=u[j]+V[j-1], odd plane O[j]=u[j]+V[j+1], u=3V) as packed fp16
ops at ~0.33/0.60 ns/elem, the device output layout is
[rowpair, img, (r, phase, c)] so each partition emits 8 KB contiguous
DRAM runs (big DMA packets), and the HOST interleaves phases during
the (already required) fp16->f32 upcast.

  TensorE : pass 1 (vertical)  V = A.T @ X, fp16 in, PSUM f32. A is
            the banded [128, 256] polyphase matrix carrying the 1/64
            scale; PSUM partition p = output rows 2p, 2p+1.
  ACT     : vh = fp16(V) PSUM->SBUF, reshuffled (eo i w)->(i eo w) so
            later APs collapse to 3D; plus the two seam columns.
  DVE     : u = 3*vh (TensorScalar, 2x mode) and the two planar
            tensor_add ops (packed fp16, 2x mode).
  GPSIMD  : input DMA doorbells only — prefetch never blocks behind
            output-tile waits.
  SYNC    : all output DMA doorbells — its o-tile waits never stall a
            compute engine.

Sharding: pure data parallel, 2 examples (256 channel-images) per core.
"""

import numpy as np

H = 128
W = 128
N_CORES = 8
EX_PER_CORE = 2
NIMG_PER_CORE = EX_PER_CORE * 128  # 256 channel-images
GROUP = 8          # images per group (matmul free dim 2x512, 1 MB out DMA)
SLAB = 32          # images per input DMA: 8 KB contiguous per partition


def _filter_matrix() -> np.ndarray:
    """A[h, m]: m in 0..127 -> even output row 2m; m in 128..255 -> odd row
    2(m-128)+1. Carries the full 1/64 scale of the separable pass."""
    A = np.zeros((H, 2 * H), np.float32)
    for i in range(H):
        # even output row 2i = (1*x[i-1] + 3*x[i])/64
        if i - 1 >= 0:
            A[i - 1, i] = 1.0 / 64
        A[i, i] = 3.0 / 64
        # odd output row 2i+1 = (3*x[i] + 1*x[i+1])/64
        A[i, H + i] = 3.0 / 64
        if i + 1 < H:
            A[i + 1, H + i] = 1.0 / 64
    return A


def filter_input() -> np.ndarray:
    return _filter_matrix().astype(np.float16)


def build_kernel_body(tc, x, filt, out, nimg):
    """Emit the kernel IR.

    x    [128(h), nimg, 128(w)] fp16 (host pre-transposed)
    filt [128, 256] fp16
    out  [128(rowpair), nimg, 512=(r, ph, c)] fp16 (host unshards)
    """
    from contextlib import ExitStack

    import concourse.mybir as mybir

    f32 = mybir.dt.float32
    f16 = mybir.dt.float16
    mult = mybir.AluOpType.mult
    add = mybir.AluOpType.add
    nc = tc.nc
    GW = GROUP * W  # 1024
    IR = 2 * GROUP  # (img, row-phase) pairs per partition
    ngroups = nimg // GROUP

    with ExitStack() as ctx:
        const_pool = ctx.enter_context(tc.tile_pool(name="const", bufs=1))
        xfirst_pool = ctx.enter_context(tc.tile_pool(name="xfirst", bufs=4))
        xin_pool = ctx.enter_context(tc.tile_pool(name="xin", bufs=2))
        v_pool = ctx.enter_context(tc.tile_pool(name="v", bufs=2, space="PSUM"))
        vh_pool = ctx.enter_context(tc.tile_pool(name="vh", bufs=4))
        u_pool = ctx.enter_context(tc.tile_pool(name="u", bufs=4))
        o_pool = ctx.enter_context(tc.tile_pool(name="o", bufs=6))

        A = const_pool.tile([128, 256], f16)
        nc.gpsimd.dma_start(A[:], filt)

        # input doorbells on GPSIMD: nothing else runs there, so prefetch
        # is never blocked behind output-tile waits. The first 4 groups
        # load per-group (short first-matmul latency); the rest in
        # 32-image slabs -> 8 KB contiguous per partition (big packets).
        gps = SLAB // GROUP  # groups per input slab
        for g in range(ngroups):
            i0 = g * GROUP
            if g < gps:
                xg_t = xfirst_pool.tile([128, GW], f16)
                nc.gpsimd.dma_start(
                    xg_t[:].rearrange("p (i w) -> p i w", i=GROUP),
                    x[:, i0 : i0 + GROUP, :],
                )
                xg = xg_t[:]
            else:
                if (g - gps) % gps == 0:
                    xs = xin_pool.tile([128, SLAB * W], f16)
                    nc.gpsimd.dma_start(
                        xs[:].rearrange("p (i w) -> p i w", i=SLAB),
                        x[:, i0 : i0 + SLAB, :],
                    )
                xg = xs[:, ((g - gps) % gps) * GW : ((g - gps) % gps + 1) * GW]

            # pass 1 (vertical) on TensorE; partition p of v holds:
            #   cols 0:1024    = V[2p,   (img, w)]  (even row phase)
            #   cols 1024:2048 = V[2p+1, (img, w)]  (odd row phase)
            # (each matmul output = one 2 KB PSUM bank = 512 f32)
            v = v_pool.tile([128, 2 * GW], f32)
            for half in range(2):
                c0 = half * 512
                nc.tensor.matmul(
                    v[:, c0 : c0 + 512],
                    A[:, 0:128],
                    xg[:, c0 : c0 + 512],
                    start=True,
                    stop=True,
                )
                nc.tensor.matmul(
                    v[:, GW + c0 : GW + c0 + 512],
                    A[:, 128:256],
                    xg[:, c0 : c0 + 512],
                    start=True,
                    stop=True,
                )

            # fp16 round-trip through SBUF on ACT, reshuffling
            # (eo i w) -> (i eo w) so the STT/seam APs are rank-3
            vh = vh_pool.tile([128, 2 * GW], f16)
            nc.scalar.copy(
                vh[:].rearrange("p (i eo w) -> p eo i w", i=GROUP, eo=2),
                v[:].rearrange("p (eo i w) -> p eo i w", i=GROUP, eo=2),
            )
            v3 = vh[:].rearrange("p (ir w) -> p ir w", ir=IR)

            # pass 2 (horizontal), planar: o layout (i r2 ph c2).
            # Packed fp16 STT ops (DVE fast-mode eligible).
            o = o_pool.tile([128, GROUP * 2 * 2 * W], f16)
            o4 = o[:].rearrange("p (ir ph c) -> p ir ph c", ir=IR, ph=2)

            # seams: E[0] = 3V[0], O[127] = 3V[127] -> flat cols 0, 255
            # (reads vh, not u, so it runs right after the ACT copy)
            nc.scalar.mul(
                o[:].rearrange("p (ir cc) -> p ir cc", ir=IR)[:, :, 0:256:255],
                v3[:, :, 0:128:127],
                3.0,
            )

            # u = 3*V, packed fp16 on DVE (TensorScalar fast mode)
            u = u_pool.tile([128, 2 * GW], f16)
            u3 = u[:].rearrange("p (ir w) -> p ir w", ir=IR)
            nc.vector.tensor_scalar_mul(u[:], vh[:], 3.0)

            # even-col plane E[j] = u[j] + V[j-1]  (j=1..127)
            nc.vector.tensor_add(o4[:, :, 0, 1:128], u3[:, :, 1:128], v3[:, :, 0:127])
            # odd-col plane O[j] = u[j] + V[j+1]  (j=0..126)
            nc.vector.tensor_add(o4[:, :, 1, 0:127], u3[:, :, 0:127], v3[:, :, 1:128])

            # one DMA per group; out DRAM layout [rowpair, img, r, ph, c]
            # gives each partition an 8 KB contiguous run per group -> big
            # packets. All output doorbells live on SYNC: it has nothing
            # else to do, so its o-tile waits never stall a compute engine.
            dst = out[:, i0 : i0 + GROUP, :]
            nc.sync.dma_start(dst, o[:].rearrange("p (i cc) -> p i cc", i=GROUP))


def build_bass(nimg=NIMG_PER_CORE, enable_asserts=False):
    import concourse.bacc as bacc
    import concourse.mybir as mybir
    import concourse.tile as tile

    f16 = mybir.dt.float16
    nc = bacc.Bacc(
        "TRN2",
        target_bir_lowering=False,
        debug=False,
        enable_asserts=enable_asserts,
        num_devices=N_CORES,
    )
    x = nc.dram_tensor("x", [H, nimg, W], f16, kind="ExternalInput").ap()
    filt = nc.dram_tensor("filt", [H, 2 * H], f16, kind="ExternalInput").ap()
    # [rowpair p, img, (r ph c)]: row = 2p+r, col = 2c+ph (host interleaves)
    out = nc.dram_tensor(
        "out", [H, nimg, 2 * 2 * W], f16, kind="ExternalOutput"
    ).ap()
    with tile.TileContext(nc) as tc:
        build_kernel_body(tc, x, filt, out, nimg)
    nc.compile()
    return nc


_NC_CACHE = {}


def kernel(x: np.ndarray, _trace=False, _trace_cores=None) -> np.ndarray:
    from concourse.bass_utils import run_bass_kernel_spmd

    x = np.asarray(x)
    assert x.shape == (16, 128, H, W), x.shape
    # fp16 downcast + per-core transpose to [h, img, w] for contiguous
    # 2 KB-per-partition input DMA lines
    x16 = x.astype(np.float16).reshape(N_CORES, NIMG_PER_CORE, H, W)
    x16 = np.ascontiguousarray(x16.transpose(0, 2, 1, 3))  # [8, h, img, w]
    A = filter_input()
    in_maps = [{"x": x16[k], "filt": A} for k in range(N_CORES)]

    key = NIMG_PER_CORE
    if key not in _NC_CACHE:
        _NC_CACHE[key] = build_bass()
    nc = _NC_CACHE[key]

    res = run_bass_kernel_spmd(
        nc,
        in_maps,
        core_ids=list(range(N_CORES)),
        trace=_trace,
        trace_cores=_trace_cores,
    )
    # [8, 128(p), 256(img), 512] fp16 where the 512 = (r, ph, c2):
    # row = 2p + r, col = 2*c2 + ph -> unshard/interleave on host
    outs = np.stack([r["out"] for r in res.results])
    out = outs.reshape(N_CORES, H, NIMG_PER_CORE, 2, 2, W).astype(np.float32)
    out = out.transpose(0, 2, 1, 3, 5, 4)  # [core, img, p, r, c2, ph]
    out = np.ascontiguousarray(out).reshape(16, 128, 2 * H, 2 * W)
    if _trace:
        kernel._last_result = res
    return out


# revision 35
# speedup vs baseline: 1.0662x; 1.0662x over previous
"""Trainium2 Bass kernel for nn_Blur (upfirdn2d: up=2, pad=(2,1,2,1), 4-tap
separable filter [1,3,3,1] x [1,3,3,1] / 64).

Input  x [16, 128, 128, 128] f32  ->  Output [16, 128, 256, 256] f32.

Math (polyphase decomposition of the zero-insertion upsample + conv):
  per axis, even outputs:  y[2i]   = (1*x[i-1] + 3*x[i]) / 8
            odd  outputs:  y[2i+1] = (3*x[i]   + 1*x[i+1]) / 8

The kernel is HBM-bandwidth bound (the 16 per-core DMA engines cap at
~360 GB/s aggregate), so all device I/O is fp16 (taps 1/64, 3/64 are
exact in fp16; quantization error ~4e-4 rel, well inside the 2e-2
gate): 8.4 MB in + 33.5 MB out per core, ~131 us of DMA-engine packet
time.

DVE runs its 2x fast modes only when every operand is 2-byte,
SBUF-resident, and packed (innermost stride 1) — interleaved stride-2
column writes disqualify it and cost ~1.2-1.4 ns/elem. So the
horizontal pass writes PLANAR column phases (even plane
E[j]=u[j]+V[j-1], odd plane O[j]=u[j]+V[j+1], u=3V) as packed fp16
ops at ~0.33/0.60 ns/elem, the device output layout is
[rowpair, img, (r, phase, c)] so each partition emits 8 KB contiguous
DRAM runs (big DMA packets), and the HOST interleaves phases during
the (already required) fp16->f32 upcast.

  TensorE : pass 1 (vertical)  V = A.T @ X, fp16 in, PSUM f32. A is
            the banded [128, 256] polyphase matrix carrying the 1/64
            scale; PSUM partition p = output rows 2p, 2p+1.
  ACT     : vh = fp16(V) PSUM->SBUF, reshuffled (eo i w)->(i eo w) so
            later APs collapse to 3D; plus the two seam columns.
  DVE     : u = 3*vh (TensorScalar, 2x mode) and the two planar
            tensor_add ops (packed fp16, 2x mode).
  GPSIMD  : input DMA doorbells only — prefetch never blocks behind
            output-tile waits.
  SYNC    : all output DMA doorbells — its o-tile waits never stall a
            compute engine.

Sharding: pure data parallel, 2 examples (256 channel-images) per core.
"""

import numpy as np

H = 128
W = 128
N_CORES = 8
EX_PER_CORE = 2
NIMG_PER_CORE = EX_PER_CORE * 128  # 256 channel-images
GROUP = 8          # images per group (matmul free dim 2x512, 1 MB out DMA)
SLAB = 32          # images per input DMA: 8 KB contiguous per partition


def _filter_matrix() -> np.ndarray:
    """A[h, m]: m in 0..127 -> even output row 2m; m in 128..255 -> odd row
    2(m-128)+1. Carries the full 1/64 scale of the separable pass."""
    A = np.zeros((H, 2 * H), np.float32)
    for i in range(H):
        # even output row 2i = (1*x[i-1] + 3*x[i])/64
        if i - 1 >= 0:
            A[i - 1, i] = 1.0 / 64
        A[i, i] = 3.0 / 64
        # odd output row 2i+1 = (3*x[i] + 1*x[i+1])/64
        A[i, H + i] = 3.0 / 64
        if i + 1 < H:
            A[i + 1, H + i] = 1.0 / 64
    return A


def filter_input() -> np.ndarray:
    return _filter_matrix().astype(np.float16)


def build_kernel_body(tc, x, filt, out, nimg):
    """Emit the kernel IR.

    x    [128(h), nimg, 128(w)] fp16 (host pre-transposed)
    filt [128, 256] fp16
    out  [128(rowpair), nimg, 512=(r, ph, c)] fp16 (host unshards)
    """
    from contextlib import ExitStack

    import concourse.mybir as mybir

    f32 = mybir.dt.float32
    f16 = mybir.dt.float16
    mult = mybir.AluOpType.mult
    add = mybir.AluOpType.add
    nc = tc.nc
    GW = GROUP * W  # 1024
    IR = 2 * GROUP  # (img, row-phase) pairs per partition
    ngroups = nimg // GROUP

    with ExitStack() as ctx:
        const_pool = ctx.enter_context(tc.tile_pool(name="const", bufs=1))
        xin_pool = ctx.enter_context(tc.tile_pool(name="xin", bufs=8))
        v_pool = ctx.enter_context(tc.tile_pool(name="v", bufs=2, space="PSUM"))
        vh_pool = ctx.enter_context(tc.tile_pool(name="vh", bufs=4))
        u_pool = ctx.enter_context(tc.tile_pool(name="u", bufs=4))
        o_pool = ctx.enter_context(tc.tile_pool(name="o", bufs=6))

        A = const_pool.tile([128, 256], f16)
        nc.gpsimd.dma_start(A[:], filt)

        # input doorbells on GPSIMD: nothing else runs there, so prefetch
        # is never blocked behind output-tile waits. 32-image slabs ->
        # 8 KB contiguous per partition (big packets).
        gps = SLAB // GROUP  # groups per input slab
        for g in range(ngroups):
            i0 = g * GROUP
            if g % gps == 0:
                xs = xin_pool.tile([128, SLAB * W], f16)
                nc.gpsimd.dma_start(
                    xs[:].rearrange("p (i w) -> p i w", i=SLAB),
                    x[:, i0 : i0 + SLAB, :],
                )
            xg = xs[:, (g % gps) * GW : (g % gps + 1) * GW]

            # pass 1 (vertical) on TensorE; partition p of v holds:
            #   cols 0:1024    = V[2p,   (img, w)]  (even row phase)
            #   cols 1024:2048 = V[2p+1, (img, w)]  (odd row phase)
            # (each matmul output = one 2 KB PSUM bank = 512 f32)
            v = v_pool.tile([128, 2 * GW], f32)
            for half in range(2):
                c0 = half * 512
                nc.tensor.matmul(
                    v[:, c0 : c0 + 512],
                    A[:, 0:128],
                    xg[:, c0 : c0 + 512],
                    start=True,
                    stop=True,
                )
                nc.tensor.matmul(
                    v[:, GW + c0 : GW + c0 + 512],
                    A[:, 128:256],
                    xg[:, c0 : c0 + 512],
                    start=True,
                    stop=True,
                )

            # fp16 round-trip through SBUF on ACT, reshuffling
            # (eo i w) -> (i eo w) so the STT/seam APs are rank-3
            vh = vh_pool.tile([128, 2 * GW], f16)
            nc.scalar.copy(
                vh[:].rearrange("p (i eo w) -> p eo i w", i=GROUP, eo=2),
                v[:].rearrange("p (eo i w) -> p eo i w", i=GROUP, eo=2),
            )
            v3 = vh[:].rearrange("p (ir w) -> p ir w", ir=IR)

            # pass 2 (horizontal), planar: o layout (i r2 ph c2).
            # Packed fp16 STT ops (DVE fast-mode eligible).
            o = o_pool.tile([128, GROUP * 2 * 2 * W], f16)
            o4 = o[:].rearrange("p (ir ph c) -> p ir ph c", ir=IR, ph=2)

            # seams: E[0] = 3V[0], O[127] = 3V[127] -> flat cols 0, 255
            # (reads vh, not u, so it runs right after the ACT copy)
            nc.scalar.mul(
                o[:].rearrange("p (ir cc) -> p ir cc", ir=IR)[:, :, 0:256:255],
                v3[:, :, 0:128:127],
                3.0,
            )

            # u = 3*V, packed fp16; ACT takes the tail quarter to shave
            # the DVE critical path
            u = u_pool.tile([128, 2 * GW], f16)
            u3 = u[:].rearrange("p (ir w) -> p ir w", ir=IR)
            nc.vector.tensor_scalar_mul(u3[:, :, 0:96], v3[:, :, 0:96], 3.0)
            nc.scalar.mul(u3[:, :, 96:128], v3[:, :, 96:128], 3.0)

            # even-col plane E[j] = u[j] + V[j-1]  (j=1..127)
            nc.vector.tensor_add(o4[:, :, 0, 1:128], u3[:, :, 1:128], v3[:, :, 0:127])
            # odd-col plane O[j] = u[j] + V[j+1]  (j=0..126)
            nc.vector.tensor_add(o4[:, :, 1, 0:127], u3[:, :, 0:127], v3[:, :, 1:128])

            # one DMA per group; out DRAM layout [rowpair, img, r, ph, c]
            # gives each partition an 8 KB contiguous run per group -> big
            # packets. All output doorbells live on SYNC: it has nothing
            # else to do, so its o-tile waits never stall a compute engine.
            dst = out[:, i0 : i0 + GROUP, :]
            nc.sync.dma_start(dst, o[:].rearrange("p (i cc) -> p i cc", i=GROUP))


def build_bass(nimg=NIMG_PER_CORE, enable_asserts=False):
    import concourse.bacc as bacc
    import concourse.mybir as mybir
    import concourse.tile as tile

    f16 = mybir.dt.float16
    nc = bacc.Bacc(
        "TRN2",
        target_bir_lowering=False,
        debug=False,
        enable_asserts=enable_asserts,
        num_devices=N_CORES,
    )
    x = nc.dram_tensor("x", [H, nimg, W], f16, kind="ExternalInput").ap()
    filt = nc.dram_tensor("filt", [H, 2 * H], f16, kind="ExternalInput").ap()
    # [rowpair p, img, (r ph c)]: row = 2p+r, col = 2c+ph (host interleaves)
    out = nc.dram_tensor(
        "out", [H, nimg, 2 * 2 * W], f16, kind="ExternalOutput"
    ).ap()
    with tile.TileContext(nc) as tc:
        build_kernel_body(tc, x, filt, out, nimg)
    nc.compile()
    return nc


_NC_CACHE = {}


def kernel(x: np.ndarray, _trace=False, _trace_cores=None) -> np.ndarray:
    from concourse.bass_utils import run_bass_kernel_spmd

    x = np.asarray(x)
    assert x.shape == (16, 128, H, W), x.shape
    # fp16 downcast + per-core transpose to [h, img, w] for contiguous
    # 2 KB-per-partition input DMA lines
    x16 = x.astype(np.float16).reshape(N_CORES, NIMG_PER_CORE, H, W)
    x16 = np.ascontiguousarray(x16.transpose(0, 2, 1, 3))  # [8, h, img, w]
    A = filter_input()
    in_maps = [{"x": x16[k], "filt": A} for k in range(N_CORES)]

    key = NIMG_PER_CORE
    if key not in _NC_CACHE:
        _NC_CACHE[key] = build_bass()
    nc = _NC_CACHE[key]

    res = run_bass_kernel_spmd(
        nc,
        in_maps,
        core_ids=list(range(N_CORES)),
        trace=_trace,
        trace_cores=_trace_cores,
    )
    # [8, 128(p), 256(img), 512] fp16 where the 512 = (r, ph, c2):
    # row = 2p + r, col = 2*c2 + ph -> unshard/interleave on host
    outs = np.stack([r["out"] for r in res.results])
    out = outs.reshape(N_CORES, H, NIMG_PER_CORE, 2, 2, W).astype(np.float32)
    out = out.transpose(0, 2, 1, 3, 5, 4)  # [core, img, p, r, c2, ph]
    out = np.ascontiguousarray(out).reshape(16, 128, 2 * H, 2 * W)
    if _trace:
        kernel._last_result = res
    return out
